# revision 27
# baseline (speedup 1.0000x reference)
"""Trainium2 Bass kernel for nn_Attention_27255862460439.

Dense transformer block: qkv projection (+rank-4 LoRA on q and v),
16-head attention over [B=4, N=2048, C=1024], output projection + bias.

Sharding: tensor-parallel over heads across 8 NeuronCores. Each core owns
2 heads (128 of the 1024 channels of q/k/v and 128 rows of w_proj) and
computes a full [8192, 1024] partial of the output projection; the host
sums the 8 partials and adds the bias.

Device-side design (per core), v2 (rebuilt after HW microbenchmarks):
 - All matmul inputs bf16 (PE runs 4x slower on fp32); PSUM accumulates fp32.
 - LoRA is folded into the qkv weights on the host: x@A@B == x@(A@B).
 - HW facts this layout is built on (measured, not simulated):
     * a matmul with K<128 partitions streams at HALF rate (427ns for
       K=64/N=512 vs 213ns for K=128/N=512), so every matmul here keeps
       128 live partitions;
     * stationary-weight reload is fully hidden even at N=65 with new
       weights every instruction (31.9ns per K=128/M=128/N=65 matmul);
     * fp8 DoubleRow is 2x FLOPs but every fp8 placement costs >=1e-2
       relative error (budget 2e-2) - not used.
 - scores: k is stored per-head ZERO-PADDED to 128 partitions (head0 in
   rows 0:64 + zero rows, head1 in rows 64:128 + zero rows).  One
   full-rate K=128 matmul per head per kt tile gives s_h[kt,q]; the
   stacked q2T [q_h0 rows | q_h1 rows] is the shared rhs (the zero rows
   of k null out the other head's q rows).
 - softmax without max-subtraction (logits are O(3) here), exp on the
   scalar engine PSUM->SBUF bf16, both heads in one [128,1024] instr.
 - attnV TOKEN-MAJOR: out[q,d] += es[kt,qchunk].T @ v[kt, d+ones]; 8
   small matmuls (4 q-chunks x 2 heads, N=65) per kt, all K=128/M=128.
   The ones column yields the softmax denominator per (q,head) ON THE
   TOKEN'S OWN PARTITION, so normalization is a DVE reciprocal [128,2]
   plus two tensor_scalar_muls with a per-partition scalar AP - no
   partition broadcast, no gpsimd at all.
 - the normalized ao2[q, feat] chunk is transposed back to feature-major
   for the proj contraction with a single PE matmul against a constant
   identity (out = ao2.T @ I, 53ns), evicted bf16.
 - proj is computed token-major (out[tok,ch] = aoT.T @ w_proj) so the
   output DMA is contiguous.
 - The attention inner loop is software-pipelined one kt ahead (the PE
   stream is in-order; scores(kt+1) must be emitted before attnV(kt)
   which waits on exp(kt)), and qkv work for batch b+1 / proj work for
   batch b-1 is interleaved between scores and attnV as PE filler during
   the exp latency, so the scalar engine (the attention-phase
   co-bottleneck) never starves.
 - Output partials are bf16 (halves the output HBM traffic; the host
   sum upcasts; costs ~1e-3 relative).
 - The last batch's qt0 proj chunks are held back as tail filler to
   cover the final normalize chain (the PE otherwise drains at the end).
"""

import sys

if '/opt/trn_rl_repo' not in sys.path:
    sys.path.insert(0, '/opt/trn_rl_repo')

import numpy as np
import ml_dtypes

import concourse.tile as tile
from concourse import bacc, mybir
from concourse.bass_utils import run_bass_kernel_spmd

BF16 = ml_dtypes.bfloat16

# Problem dims (hardcoded per contract)
B, N, C, H, D = 4, 2048, 1024, 16, 64
T = B * N                  # 8192 tokens
NCORES = 8
HC = H // NCORES           # 2 heads per core
LC = HC * D                # 128 local qkv columns / proj rows per core
KO = C // 128              # 8 contraction subtiles
SCALE = D ** -0.5          # 0.125
NKT = N // 128             # 16 ktok tiles per batch
DT = mybir.dt

# Schraudolph bf16 exp: bits16(es) = round(A16*logit + B16) with
# B16 = 127*2^7 - C16.  The round+truncate is obtained for free by letting
# the DVE add 1.5*2^23 + B16 to the (A16-prescaled) scores in fp32: the
# result's low halfword IS the bf16 pattern of exp(logit).  A16 is folded
# into the q eviction; the ACT-path exp undoes it via its free affine
# (activation scale = 1/A16).  C16=7 tuned for zero-mean relative error.
# SCHR_KT kt tiles run on the DVE instead of ACT to balance the two
# engines (ACT exp is 1136ns/tile on HW and otherwise paces the loop);
# attnV reads the fp32 staging buffer through a stride-2 bf16 view, which
# a microbenchmark showed costs nothing extra even for stationary weights.
A16 = 128.0 / float(np.log(2.0))
C16 = 7.0
B_MAGIC = float(np.float32(1.5 * 2 ** 23 + (127.0 * 2 ** 7 - C16)))
INV_A16 = 1.0 / A16
SCHR_KT = frozenset({5, 11})


def _build_nc(loop_n: int = 1):
    nc = bacc.Bacc(None, target_bir_lowering=False, debug=False)
    xT_d = nc.dram_tensor('xT', [C, T], DT.bfloat16, kind='ExternalInput')
    wq_d = nc.dram_tensor('wq', [128, KO, LC], DT.bfloat16, kind='ExternalInput')
    wk_d = nc.dram_tensor('wk', [128, KO, LC], DT.bfloat16, kind='ExternalInput')
    wv_d = nc.dram_tensor('wv', [128, KO, LC], DT.bfloat16, kind='ExternalInput')
    wp_d = nc.dram_tensor('wp', [LC, C], DT.bfloat16, kind='ExternalInput')
    id_d = nc.dram_tensor('ident', [128, 128], DT.bfloat16, kind='ExternalInput')
    out_d = nc.dram_tensor('out', [T, C], DT.bfloat16, kind='ExternalOutput')

    xT_r = xT_d.rearrange("(ko p) t -> p ko t", p=128)

    with tile.TileContext(nc) as tc:
        with (
            tc.tile_pool(name='weights', bufs=1) as cw,
            tc.tile_pool(name='xin', bufs=6) as xp,
            tc.tile_pool(name='slabs', bufs=1) as slabs,
            tc.tile_pool(name='work', bufs=1) as wk_pool,
            tc.tile_pool(name='ps', bufs=1, space='PSUM') as psp,
        ):
            # Half-granularity loads for the tensors on the prologue
            # critical path: the first q/k matmul chains only need ko 0..3,
            # so they start as soon as the first halves land.
            h = KO // 2
            xt0 = cw.tile([128, KO, 512], DT.bfloat16, name='xt0')
            wq_sb = cw.tile([128, KO, LC], DT.bfloat16)
            wk_sb = cw.tile([128, KO, LC], DT.bfloat16)
            nc.sync.dma_start(xt0[:, 0:h, :], xT_r[:, 0:h, 0:512])
            nc.sync.dma_start(wq_sb[:, 0:h, :], wq_d[:, 0:h, :])
            nc.sync.dma_start(wk_sb[:, 0:h, :], wk_d[:, 0:h, :])
            nc.sync.dma_start(xt0[:, h:KO, :], xT_r[:, h:KO, 0:512])
            nc.sync.dma_start(wq_sb[:, h:KO, :], wq_d[:, h:KO, :])
            nc.sync.dma_start(wk_sb[:, h:KO, :], wk_d[:, h:KO, :])
            wv_sb = cw.tile([128, KO, LC], DT.bfloat16)
            wp_sb = cw.tile([LC, C], DT.bfloat16)
            id_sb = cw.tile([128, 128], DT.bfloat16, name='id_sb')
            nc.sync.dma_start(wv_sb[:], wv_d[:])
            nc.sync.dma_start(wp_sb[:], wp_d[:])
            nc.sync.dma_start(id_sb[:], id_d[:])

            # Preload the exp table set while the first DMAs are in flight
            # (saves the ~2.7us ACT_TABLE_LOAD from the critical path).
            warm = cw.tile([1, 8], DT.float32)
            nc.any.memset(warm[:], 0.0)
            nc.scalar.activation(warm[:], warm[:],
                                 mybir.ActivationFunctionType.Exp)

            # Fine-grained persistent tiles (exact producer/consumer regions).
            q2T = [slabs.tile([128, 512], DT.bfloat16, name=f'q2T_{g}')
                   for g in range(T // 512)]
            # per-head k, zero-padded to 128 partitions: k0 rows 0:64 hold
            # head0's d, rows 64:128 stay zero; k1 is the mirror image.
            # The zero rows are memset once and never rewritten.
            k0T = [slabs.tile([128, 512], DT.bfloat16, name=f'k0T_{g}')
                   for g in range(T // 512)]
            k1T = [slabs.tile([128, 512], DT.bfloat16, name=f'k1T_{g}')
                   for g in range(T // 512)]
            for kt_ in k0T:
                nc.any.memset(kt_[64:128, :], 0.0)
            for kt_ in k1T:
                nc.any.memset(kt_[0:64, :], 0.0)
            aoT = [slabs.tile([128, 512], DT.bfloat16, name=f'aoT_{g}')
                   for g in range(T // 512)]
            vtok = [slabs.tile([128, 130], DT.bfloat16, name=f'vtok_{k}')
                    for k in range(T // 128)]
            for vt in vtok:
                nc.any.memset(vt[:, 64:65], 1.0)
                nc.any.memset(vt[:, 129:130], 1.0)
            # zero weights for the per-qt PSUM bank-clearing matmul (a
            # start=True matmul zeroes its whole 2KB bank, so banks shared
            # by several accumulation regions must be cleared by ONE
            # full-bank matmul; the regions then accumulate start=False).
            zpad = slabs.tile([128, 128], DT.bfloat16, name='zpad')
            nc.any.memset(zpad[:], 0.0)

            def emit_body():
                # ---------- phase emitters ----------
                def qkv_tile_chunks(b, tt):
                    """Filler chunks (closures) computing q/k/v for one
                    512-token tile. Each chunk is a small burst of PE work."""
                    g = b * 4 + tt
                    sl = slice(g * 512, (g + 1) * 512)
                    state = {}

                    def load_x():
                        if g == 0:
                            state['xt'] = xt0
                            return
                        xt = xp.tile([128, KO, 512], DT.bfloat16, tag='xt')
                        nc.sync.dma_start(xt[:], xT_r[:, :, sl])
                        state['xt'] = xt

                    def q_mms(half):
                        if half == 0:
                            state['psq'] = psp.tile([128, 512], DT.float32,
                                                    tag='aux', bufs=2, name='psq')
                        psq, xt = state['psq'], state['xt']
                        for ko in range(half * 4, half * 4 + 4):
                            nc.tensor.matmul(psq[:], wq_sb[:, ko, :], xt[:, ko, :],
                                             start=(ko == 0), stop=(ko == KO - 1))
                        if half == 1:
                            nc.vector.tensor_scalar_mul(q2T[g][:], psq[:],
                                                        SCALE * A16)

                    def k_mms(half):
                        if half == 0:
                            state['psk'] = psp.tile([128, 512], DT.float32,
                                                    tag='aux', bufs=2, name='psk')
                        psk, xt = state['psk'], state['xt']
                        for ko in range(half * 4, half * 4 + 4):
                            nc.tensor.matmul(psk[:], wk_sb[:, ko, :], xt[:, ko, :],
                                             start=(ko == 0), stop=(ko == KO - 1))
                        if half == 1:
                            # per-head zero-padded eviction (see header)
                            nc.vector.tensor_copy(k0T[g][0:64, :], psk[0:64, :])
                            nc.vector.tensor_copy(k1T[g][64:128, :],
                                                  psk[64:128, :])

                    def v_mms(sub):
                        xt = state['xt']
                        psv = psp.tile([128, 128], DT.float32, tag='aux', bufs=2,
                                       name='psv')
                        tsl = slice(sub * 128, (sub + 1) * 128)
                        for ko in range(KO):
                            nc.tensor.matmul(psv[:], xt[:, ko, tsl],
                                             wv_sb[:, ko, :],
                                             start=(ko == 0), stop=(ko == KO - 1))
                        kt = g * 4 + sub
                        # one strided copy: [128,2,64] -> cols {0:64, 65:129}
                        dst = vtok[kt].rearrange("p (two c) -> p two c", two=2)
                        src = psv[:].rearrange("p (two c) -> p two c", two=2)
                        nc.vector.tensor_copy(dst[:, :, 0:64], src)

                    chunks = [load_x,
                              lambda: q_mms(0), lambda: q_mms(1),
                              lambda: k_mms(0), lambda: k_mms(1)]
                    chunks += [(lambda s: lambda: v_mms(s))(s) for s in range(4)]
                    return chunks

                def proj_tile_chunk(b, tt, ch):
                    """One proj output tile: 1 matmul + evict + DMA out."""
                    tg = b * 16 + tt
                    tsl = slice(tg * 128, (tg + 1) * 128)
                    ao_tile = aoT[tg // 4]
                    asl = slice((tg % 4) * 128, (tg % 4 + 1) * 128)
                    csl = slice(ch * 512, (ch + 1) * 512)

                    tail = (b == B - 1 and tt >= 12)

                    def run():
                        pspj = psp.tile([128, 512], DT.float32, tag='aux', bufs=2,
                                        name='pspj')
                        nc.tensor.matmul(pspj[:], ao_tile[:, asl], wp_sb[:, csl],
                                         start=True, stop=True)
                        ob = wk_pool.tile([128, 512], DT.bfloat16, tag='ob', bufs=4)
                        # All mid-kernel evictions on DVE: an ACT Copy
                        # interleaved with Exp activations forces an ACT
                        # table reload (~2.7us) per switch.  Only the kernel
                        # tail (no Exps after it) may use ACT.
                        if tail and ch == 1:
                            nc.scalar.copy(ob[:], pspj[:])
                        else:
                            nc.vector.tensor_copy(ob[:], pspj[:])
                        nc.sync.dma_start(out_d[tsl, csl], ob[:])
                    return run

                def emit_scores(b, qt, kt16):
                    ktg = b * NKT + kt16
                    ksl = slice((ktg % 4) * 128, (ktg % 4 + 1) * 128)
                    qg = b * 4 + qt
                    ss = psp.tile([128, 1024], DT.float32, tag='scores', bufs=2,
                                  name='ss')
                    # full-rate K=128 per head (zero-padded k); out s_h[kt,q]
                    nc.tensor.matmul(ss[:, 0:512], k0T[ktg // 4][:, ksl],
                                     q2T[qg][:], start=True, stop=True)
                    nc.tensor.matmul(ss[:, 512:1024], k1T[ktg // 4][:, ksl],
                                     q2T[qg][:], start=True, stop=True)
                    return ss

                # ---------- prologue: qkv for batch 0, tile 0 ----------
                tail_fill = []
                for c in qkv_tile_chunks(0, 0):
                    c()

                # ---------- attention per batch, with filler interleave ----
                # Fillers are (deadline_tile_or_None, closure). Deadlined
                # chunks (batch-0 qkv tiles 1..3) must be EMITTED before the
                # scores that read their outputs; the rest are paced evenly.
                for b in range(B):
                    fillers = []
                    if b == 0:
                        for tt in range(1, 4):
                            for c in qkv_tile_chunks(0, tt):
                                fillers.append((tt, c))
                    if b + 1 < B:
                        for tt in range(4):
                            for c in qkv_tile_chunks(b + 1, tt):
                                fillers.append((None, c))
                    if b > 0:
                        # leftover proj chunks from batch b-1 (its qt=3)
                        for tt in range(12, 16):
                            for ch in range(2):
                                fillers.append((None, proj_tile_chunk(b - 1, tt, ch)))
                    fillers.reverse()          # pop() takes from the front

                    n_iters = 4 * NKT
                    it = 0
                    if b == 0:
                        ss_next = emit_scores(0, 0, 0)
                    for qt in range(4):
                        # token-major attnV accumulators: chunks {0,1} in
                        # bank A, {2,3} in bank B; per chunk the 130 columns
                        # are [64 d_h0 | den_h0 | 64 d_h1 | den_h1].
                        psoA = psp.tile([128, 512], DT.float32, tag='oA',
                                        bufs=1, name='psoA')
                        psoB = psp.tile([128, 512], DT.float32, tag='oB',
                                        bufs=1, name='psoB')
                        obank = [psoA, psoA, psoB, psoB]

                        def o_region(c, h):
                            cb = (c % 2) * 260
                            return obank[c][:, cb + h * 65: cb + (h + 1) * 65]

                        def attn_v(ktg_, es_sl_, last_):
                            # token-major attnV: 8 small full-K matmuls
                            for cch in range(4):
                                for hh in range(2):
                                    nc.tensor.matmul(
                                        o_region(cch, hh),
                                        es_sl_(hh * 512 + cch * 128,
                                               hh * 512 + (cch + 1) * 128),
                                        vtok[ktg_][:, hh * 65:(hh + 1) * 65],
                                        start=False,
                                        stop=last_,
                                        skip_group_check=True)

                        # attnV runs one iteration BEHIND exp: by the time
                        # attnV(kt) is issued, exp(kt) finished during the
                        # previous iteration's PE work, so the PE never
                        # blocks on the scalar engine's latency.
                        pending = None
                        for kt16 in range(NKT):
                            ktg = b * NKT + kt16
                            # mandatory flush: producers of the tile the
                            # upcoming scores emission will read
                            next_tile = min(qt * NKT + kt16 + 1, n_iters - 1) // 16
                            next_kt_tile = (kt16 + 1) // 4 if kt16 < NKT - 1 else 0
                            while fillers and fillers[-1][0] is not None and \
                                    fillers[-1][0] <= max(next_kt_tile, next_tile):
                                fillers.pop()[1]()
                            ss = ss_next
                            if kt16 < NKT - 1:
                                ss_next = emit_scores(b, qt, kt16 + 1)
                            elif qt < 3:
                                ss_next = emit_scores(b, qt + 1, 0)
                            elif b + 1 < B:
                                ss_next = emit_scores(b + 1, 0, 0)
                            else:
                                ss_next = None
                            if kt16 in SCHR_KT:
                                # Schraudolph path: DVE adds the magic bias;
                                # the fp32 result's low halfwords are read as
                                # a stride-2 bf16 AP by attnV directly.
                                stg = wk_pool.tile([128, 2048], DT.bfloat16,
                                                   tag='stg', bufs=4)
                                nc.vector.tensor_scalar_add(
                                    stg[:].bitcast(DT.float32), ss[:], B_MAGIC)
                                sv = stg[:].rearrange(
                                    "p (k two) -> p k two", two=2)
                                es_sl = (lambda sv: lambda a, b: sv[:, a:b, 0])(sv)
                            else:
                                es = wk_pool.tile([128, 1024], DT.bfloat16,
                                                  tag='es', bufs=8)
                                nc.scalar.activation(
                                    es[:], ss[:],
                                    mybir.ActivationFunctionType.Exp,
                                    scale=INV_A16)
                                es_sl = (lambda es: lambda a, b: es[:, a:b])(es)
                            # paced filler PE work rides out the exp latency
                            remaining = n_iters - it
                            if fillers and fillers[-1][0] is not None:
                                # deadlined (batch-0) work: pace to land just
                                # ahead of its consumers instead of bursting
                                # at the mandatory-flush point
                                remaining = max(1, min(remaining,
                                                       4 * fillers[-1][0] - it))
                            want = -(-len(fillers) // remaining)
                            for _ in range(min(want, 4)):
                                if fillers:
                                    fillers.pop()[1]()
                            it += 1
                            if kt16 == 0:
                                # clear both pso banks (see zpad comment)
                                nc.tensor.matmul(psoA[:, 0:390], zpad[:],
                                                 q2T[b * 4 + qt][:, 0:390],
                                                 start=True, stop=False,
                                                 skip_group_check=True)
                                nc.tensor.matmul(psoB[:, 0:390], zpad[:],
                                                 q2T[b * 4 + qt][:, 0:390],
                                                 start=True, stop=False,
                                                 skip_group_check=True)
                            if pending is not None:
                                pending()
                            pending = (lambda k_, e_, l_: lambda:
                                       attn_v(k_, e_, l_))(
                                           ktg, es_sl, kt16 == NKT - 1)
                        pending()
                        # normalize qt: denominators sit on each token's own
                        # partition (cols 64/129 of its chunk), so this is a
                        # reciprocal + two per-partition scalar muls, then a
                        # PE transpose (matmul vs identity) back to
                        # feature-major for proj.
                        qg = b * 4 + qt
                        for cch in range(4):
                            cb = (cch % 2) * 260
                            bank = obank[cch]
                            rr = wk_pool.tile([128, 2], DT.float32, tag='rr',
                                              bufs=4)
                            den = bank[:, cb:cb + 130].rearrange(
                                "p (two c) -> p two c", two=2)[:, :, 64]
                            nc.vector.reciprocal(rr[:], den)
                            ao2 = wk_pool.tile([128, 128], DT.bfloat16,
                                               tag='ao2', bufs=4)
                            nc.vector.tensor_scalar_mul(
                                ao2[:, 0:64], bank[:, cb:cb + 64], rr[:, 0:1])
                            nc.vector.tensor_scalar_mul(
                                ao2[:, 64:128], bank[:, cb + 65:cb + 129],
                                rr[:, 1:2])
                            psT = psp.tile([128, 128], DT.float32, tag='aux',
                                           bufs=2, name='psT')
                            nc.tensor.matmul(psT[:], ao2[:], id_sb[:],
                                             start=True, stop=True)
                            nc.vector.tensor_copy(
                                aoT[qg][:, cch * 128:(cch + 1) * 128], psT[:])
                        # this qt's proj work becomes filler for later qts.
                        # Exception: the last batch's qt=0 chunks are held
                        # back as tail filler to cover the final normalize
                        # chain (otherwise the PE drains at the end).
                        if qt < 3:
                            if b == B - 1 and qt == 0:
                                tail_fill.extend(
                                    proj_tile_chunk(b, tt, ch)
                                    for tt in range(0, 4) for ch in range(2))
                            else:
                                for tt in range(qt * 4, qt * 4 + 4):
                                    for ch in range(2):
                                        fillers.insert(
                                            0, (None, proj_tile_chunk(b, tt, ch)))
                    # leftover fillers
                    while fillers:
                        fillers.pop()[1]()

                # ---------- epilogue ----------
                # held-back qt0 chunks (dependency-free) cover the last
                # normalize chain, then the final qt=3 chunks drain.
                for c in tail_fill:
                    c()
                for tt in range(12, 16):
                    for ch in range(2):
                        proj_tile_chunk(B - 1, tt, ch)()

            if loop_n > 1:
                with tc.For_i(0, loop_n, 1):
                    emit_body()
            else:
                emit_body()

    nc.compile()
    return nc


def _prep_inputs(inputs):
    """Host-side sharding prep: returns per-core input maps."""
    x = np.asarray(inputs['x'], dtype=np.float32)
    w_qkv = np.asarray(inputs['w_qkv'], dtype=np.float32)
    w_a_q = np.asarray(inputs['w_a_q'], dtype=np.float32)
    w_b_q = np.asarray(inputs['w_b_q'], dtype=np.float32)
    w_a_v = np.asarray(inputs['w_a_v'], dtype=np.float32)
    w_b_v = np.asarray(inputs['w_b_v'], dtype=np.float32)
    w_proj = np.asarray(inputs['w_proj'], dtype=np.float32)

    wq_eff = w_qkv[:, :C] + w_a_q @ w_b_q
    wk_full = w_qkv[:, C:2 * C]
    wv_eff = w_qkv[:, 2 * C:] + w_a_v @ w_b_v

    xT = np.ascontiguousarray(x.reshape(T, C).T).astype(BF16)
    ident = np.eye(128, dtype=BF16)

    in_maps = []
    for m in range(NCORES):
        cols = slice(m * LC, (m + 1) * LC)
        def pack(w):
            # [C, LC] -> [p, ko, m] so the device DMA is one contiguous blob
            return np.ascontiguousarray(
                w.reshape(KO, 128, LC).transpose(1, 0, 2)).astype(BF16)
        in_maps.append({
            'xT': xT,
            'wq': pack(wq_eff[:, cols]),
            'wk': pack(wk_full[:, cols]),
            'wv': pack(wv_eff[:, cols]),
            'wp': np.ascontiguousarray(w_proj[cols, :]).astype(BF16),
            'ident': ident,
        })
    return in_maps


_nc_cache = None


def _get_nc():
    global _nc_cache
    if _nc_cache is None:
        _nc_cache = _build_nc()
    return _nc_cache


def kernel(**inputs) -> np.ndarray:
    nc = _get_nc()
    in_maps = _prep_inputs(inputs)
    res = run_bass_kernel_spmd(nc, in_maps, core_ids=list(range(NCORES)))
    b_proj = np.asarray(inputs['b_proj'], dtype=np.float32)
    total = res.results[0]['out'].astype(np.float32, copy=True)
    for m in range(1, NCORES):
        total += res.results[m]['out']
    total += b_proj[None, :]
    return total.reshape(B, N, C)
